# revision 7
# baseline (speedup 1.0000x reference)
"""Trainium2 Bass kernel for the ContrastiveSWM transition model (GNN message passing).

Full-input contract: kernel(**inputs) takes the unsharded inputs from
reference.setup_inputs() and returns the full [16384, 3, 128] output.
Internally: batch is sharded across 8 NeuronCores (2048 samples each), params
replicated; each core runs an identical Bass/Tile program over 4 tiles of 512
samples kept feature-major ([feature, sample]) in SBUF.

Key implementation choices:
- All matmuls run as float32r (fp32 storage, fast PE path at N=512).
- LayerNorm mean-subtraction is folded into host-pre-centered weights
  (mean(y) over features is linear in x), LN gain g folded into weights;
  variance comes from a (1/g^2)-weighted ones-matmul over squared
  activations; 1/std is broadcast across partitions on GPSIMD.
- Per-sample 3-node graph: src/tgt gather = partition-row slices of the
  feature-major state; scatter-add over edges = accumulating both edges'
  final-layer matmul groups into one PSUM tile.
- one_hot(action) is precomputed on the host into a padded input slab.
"""

import os
import sys
from contextlib import ExitStack

import numpy as np

for _p in ("/opt/trn_rl_repo", "/root/.axon_site/_ro/trn_rl_repo"):
    if os.path.isdir(_p) and _p not in sys.path:
        sys.path.insert(0, _p)

import concourse.bass as bass  # noqa: E402
import concourse.mybir as mybir  # noqa: E402
import concourse.tile as tile  # noqa: E402
from concourse import bacc  # noqa: E402

P = 128
NT = 512          # samples per tile (matmul moving free dim)
N_CORES = 8
B = 16384
BPC = B // N_CORES            # 2048 samples per core
T = BPC // NT                 # 4 tiles per core
D = 128                       # latent dim per agent
H = 512                       # hidden
A = 3                         # agents
ACT = 5                       # action dim
EPS = 1e-5
F32 = mybir.dt.float32
F32R = mybir.dt.float32r
AF = mybir.ActivationFunctionType

# edge list: ROWS=src, COLS=tgt; edges 2a, 2a+1 share src agent a
EDGE_TGTS = {0: (1, 2), 1: (0, 2), 2: (0, 1)}

# one-hot rows live at OH_OFF + a*ACT + v inside the 128-row input slab
OH_OFF = 16

_CACHE = {}


def _build_program(n_tiles: int):
    nc = bacc.Bacc("TRN2", target_bir_lowering=False, debug=False)
    BT = n_tiles * NT

    def din(name, shape, dt=F32R):
        return nc.dram_tensor(name, list(shape), dt, kind="ExternalInput").ap()

    io_d = din("io_p", [P, BT])
    # encoder
    ew1_d = din("ew1", [P, H])
    ew2_d = din("ew2", [P, 4, H])
    ew3_d = din("ew3", [P, 4, A * D])
    # edge mlp
    gw0_d = din("gw0", [P, 2, H])
    gw1_d = din("gw1", [P, 4, H])
    gw2_d = din("gw2", [P, 4, H])
    gwf_d = din("gwf", [P, 4, H])
    # node mlp
    nw0s_d = din("nw0s", [P, H])
    nw0a_d = din("nw0a", [P, A, H])
    nw0g_d = din("nw0g", [P, 4, H])
    nw1_d = din("nw1", [P, 4, H])
    nw2_d = din("nw2", [P, 4, H])
    nwf_d = din("nwf", [P, 4, D])
    # packed per-chunk column vectors [P, ncols]
    cols_d = din("cols", [P, 32], F32)
    out_d = nc.dram_tensor("out_fm", [P, A, BT], F32, kind="ExternalOutput").ap()

    # column indices in cols_d (see _pack_cols)
    CI = {}
    _ci = iter(range(64))

    def ci(name, n):
        CI[name] = next(_ci)
        for _ in range(n - 1):
            next(_ci)
        return CI[name]

    ci("eb1", 4); ci("es1", 4); ci("ebt1", 4)
    ci("eb2", 4); ci("es2", 4); ci("ebt2", 4)
    ci("eb3", 3)
    ci("gb0", 4)  # 31 cols -> fits 32
    cols2_d = din("cols2", [P, 32], F32)
    CI2 = {}
    _ci2 = iter(range(64))

    def ci2(name, n):
        CI2[name] = next(_ci2)
        for _ in range(n - 1):
            next(_ci2)
        return CI2[name]

    ci2("gb1", 4); ci2("gs1", 4); ci2("gbt1", 4)
    ci2("gb2", 4); ci2("gs2", 4); ci2("gbt2", 4)
    ci2("gbf2", 4)
    ci2("nb0", 4)  # 32 cols exactly
    cols3_d = din("cols3", [P, 32], F32)
    scols_d = din("scols", [P, 24], F32R)
    CI3 = {}
    _ci3 = iter(range(64))

    def ci3(name, n):
        CI3[name] = next(_ci3)
        for _ in range(n - 1):
            next(_ci3)
        return CI3[name]

    ci3("nb1", 4); ci3("ns1", 4); ci3("nbt1", 4)
    ci3("nb2", 4); ci3("ns2", 4); ci3("nbt2", 4)
    ci3("nbf", 1)

    with tile.TileContext(nc) as tc:
        with ExitStack() as ctx:
            wp = ctx.enter_context(tc.tile_pool(name="wp", bufs=1))
            wk = ctx.enter_context(tc.tile_pool(name="wk", bufs=2))
            pmm = ctx.enter_context(tc.tile_pool(name="pmm", bufs=6, space="PSUM"))
            psq = ctx.enter_context(tc.tile_pool(name="psq", bufs=2, space="PSUM"))

            # ---- load weights (once) ----
            def wload(ap_d, shape, name, dt=F32R):
                t = wp.tile(shape, dt, name=name, tag=name)
                nc.sync.dma_start(t[:], ap_d[:])
                return t

            ew1 = wload(ew1_d, [P, H], "ew1")
            ew2 = wload(ew2_d, [P, 4, H], "ew2")
            ew3 = wload(ew3_d, [P, 4, A * D], "ew3")
            gw0 = wload(gw0_d, [P, 2, H], "gw0")
            gw1 = wload(gw1_d, [P, 4, H], "gw1")
            gw2 = wload(gw2_d, [P, 4, H], "gw2")
            gwf = wload(gwf_d, [P, 4, H], "gwf")
            nw0s = wload(nw0s_d, [P, H], "nw0s")
            nw0a = wload(nw0a_d, [P, A, H], "nw0a")
            nw0g = wload(nw0g_d, [P, 4, H], "nw0g")
            nw1 = wload(nw1_d, [P, 4, H], "nw1")
            nw2 = wload(nw2_d, [P, 4, H], "nw2")
            nwf = wload(nwf_d, [P, 4, D], "nwf")
            cols = wload(cols_d, [P, 32], "cols", F32)
            cols2 = wload(cols2_d, [P, 32], "cols2", F32)
            cols3 = wload(cols3_d, [P, 32], "cols3", F32)
            scols = wload(scols_d, [P, 24], "scols", F32R)
            eps_t = wp.tile([1, 1], F32, name="eps_t", tag="eps_t")
            nc.vector.memset(eps_t[:], EPS)

            def col(name):
                if name in CI:
                    return cols[:, CI[name]:CI[name] + 1]
                if name in CI2:
                    return cols2[:, CI2[name]:CI2[name] + 1]
                return cols3[:, CI3[name]:CI3[name] + 1]

            SCI = {"es1": 0, "es2": 4, "gs1": 8, "gs2": 12, "ns1": 16, "ns2": 20}

            def scolq(name, m):
                i = SCI[name] + m
                return scols[:, i:i + 1]

            def colq(name, m):
                if name in CI:
                    return cols[:, CI[name] + m:CI[name] + m + 1]
                if name in CI2:
                    return cols2[:, CI2[name] + m:CI2[name] + m + 1]
                return cols3[:, CI3[name] + m:CI3[name] + m + 1]

            def mm_group(ps, w_chunks, x_chunks, start=True, stop=True):
                n = len(w_chunks)
                for k in range(n):
                    nc.tensor.matmul(
                        ps[:],
                        w_chunks[k].bitcast(F32R),
                        x_chunks[k].bitcast(F32R),
                        start=(start and k == 0),
                        stop=(stop and k == n - 1),
                    )

            def ln_relu_layer(x_chunks, w3, bname, sname, btname, out_tag, MC=4):
                """relu(LN(x @ W + b)) with host-pre-centered W; returns [P,MC,NT] quad."""
                out = wk.tile([P, MC, NT], F32R, name=f"{out_tag}", tag=out_tag)
                cb = wk.tile([P, MC, NT], F32, name="cb", tag="cbsq", bufs=3)
                sq = wk.tile([P, MC, NT], F32R, name="sq", tag="cbsq", bufs=3)
                ssq = psq.tile([1, NT], F32, name="ssq", tag="ssq")
                for m in range(MC):
                    ps = pmm.tile([P, NT], F32, name="ps", tag="mm")
                    mm_group(ps, [w3[:, k, m * P:(m + 1) * P] for k in range(len(x_chunks))],
                             x_chunks)
                    nc.vector.tensor_scalar_add(cb[:, m, :], ps[:], colq(bname, m))
                    nc.scalar.activation(sq[:, m, :], ps[:], AF.Square,
                                         bias=colq(bname, m), scale=1.0)
                    nc.tensor.matmul(ssq[:], scolq(sname, m),
                                     sq[:, m, :],
                                     start=(m == 0), stop=(m == MC - 1))
                sd = wk.tile([1, NT], F32, name="sd", tag="sd")
                nc.scalar.activation(sd[:], ssq[:], AF.Sqrt, bias=eps_t[:],
                                     scale=1.0 / H)
                nc.vector.reciprocal(sd[:], sd[:])
                ibc = wk.tile([P, NT], F32, name="ibc", tag="ibc")
                nc.gpsimd.partition_broadcast(ibc[:], sd[:])
                for m in range(MC):
                    nc.vector.tensor_mul(out=cb[:, m, :], in0=cb[:, m, :], in1=ibc[:])
                    nc.scalar.activation(out[:, m, :], cb[:, m, :], AF.Relu,
                                         bias=colq(btname, m), scale=1.0)
                return out

            def relu_layer(x_chunks, w_chunks_fn, bname, out_tag, MC=4):
                out = wk.tile([P, MC, NT], F32R, name=f"{out_tag}", tag=out_tag)
                for m in range(MC):
                    ps = pmm.tile([P, NT], F32, name="ps", tag="mm")
                    mm_group(ps, w_chunks_fn(m), x_chunks)
                    nc.scalar.activation(out[:, m, :], ps[:], AF.Relu,
                                         bias=colq(bname, m), scale=1.0)
                return out

            # ---- per-tile pipeline ----
            for t in range(n_tiles):
                io = wk.tile([P, NT], F32R, name="io", tag="io")
                nc.sync.dma_start(io[:], io_d[:, t * NT:(t + 1) * NT])

                # encoder
                h1 = ln_relu_layer([io[:]], _W1View(ew1),
                                   "eb1", "es1", "ebt1", "qa")
                h2 = ln_relu_layer([h1[:, k, :] for k in range(4)], ew2,
                                   "eb2", "es2", "ebt2", "qb")
                state = wk.tile([P, A, NT], F32R, name="state", tag="state")
                for m in range(A):
                    ps = pmm.tile([P, NT], F32, name="ps", tag="mm")
                    mm_group(ps, [ew3[:, k, m * P:(m + 1) * P] for k in range(4)],
                             [h2[:, k, :] for k in range(4)])
                    nc.vector.tensor_scalar_add(state[:, m, :], ps[:], colq("eb3", m))

                for a in range(A):
                    # two edges with src == a, accumulated into one PSUM quad
                    agg_ps = [pmm.tile([P, NT], F32, name="aggps", tag="mm")
                              for _ in range(4)]
                    for ei, tgt in enumerate(EDGE_TGTS[a]):
                        e0 = relu_layer(
                            [state[:, a, :], state[:, tgt, :]],
                            lambda m: [gw0[:, 0, m * P:(m + 1) * P],
                                       gw0[:, 1, m * P:(m + 1) * P]],
                            "gb0", "qa")
                        e1 = ln_relu_layer([e0[:, k, :] for k in range(4)], gw1,
                                           "gb1", "gs1", "gbt1", "qb")
                        e2 = ln_relu_layer([e1[:, k, :] for k in range(4)], gw2,
                                           "gb2", "gs2", "gbt2", "qc")
                        for m in range(4):
                            mm_group(agg_ps[m],
                                     [gwf[:, k, m * P:(m + 1) * P] for k in range(4)],
                                     [e2[:, k, :] for k in range(4)],
                                     start=(ei == 0), stop=(ei == 1))
                    agg = wk.tile([P, 4, NT], F32R, name="agg", tag="agg", bufs=1)
                    for m in range(4):
                        nc.vector.tensor_scalar_add(agg[:, m, :], agg_ps[m][:],
                                                    colq("gbf2", m))

                    # node mlp for agent a
                    n0 = relu_layer(
                        [state[:, a, :], io[:], *(agg[:, k, :] for k in range(4))],
                        lambda m: [nw0s[:, m * P:(m + 1) * P],
                                   nw0a[:, a, m * P:(m + 1) * P],
                                   *(nw0g[:, k, m * P:(m + 1) * P] for k in range(4))],
                        "nb0", "qa")
                    n1 = ln_relu_layer([n0[:, k, :] for k in range(4)], nw1,
                                       "nb1", "ns1", "nbt1", "qb")
                    n2 = ln_relu_layer([n1[:, k, :] for k in range(4)], nw2,
                                       "nb2", "ns2", "nbt2", "qc")
                    ps = pmm.tile([P, NT], F32, name="ps", tag="mm")
                    mm_group(ps, [nwf[:, k, :] for k in range(4)],
                             [n2[:, k, :] for k in range(4)])
                    outp = wk.tile([P, NT], F32, name="outp", tag="outp")
                    nc.vector.tensor_add(out=outp[:], in0=ps[:], in1=state[:, a, :])
                    nc.vector.tensor_scalar_add(outp[:], outp[:], col("nbf"))
                    nc.sync.dma_start(out_d[:, a, t * NT:(t + 1) * NT], outp[:])

    nc.compile()
    return nc


class _W1View:
    """Adapter so ew1 [P, H] can be indexed like a [P, 1, H] 3-tensor."""

    def __init__(self, t):
        self.t = t

    def __getitem__(self, idx):
        _, k, sl = idx
        assert k == 0
        return self.t[:, sl]


def _center_fold(W, b, g):
    Wp = (W - W.mean(axis=1, keepdims=True)) * g[None, :]
    bp = (b - b.mean()) * g
    s = 1.0 / (g * g)
    return Wp.astype(np.float32), bp.astype(np.float32), s.astype(np.float32)


def _kchunks(W):
    K, M = W.shape
    assert K % P == 0
    return np.ascontiguousarray(W.reshape(K // P, P, M).transpose(1, 0, 2))


def _colpack(vecs, width=32):
    """Pack a list of [M] vectors (M multiple of 128) into [P, width]."""
    out = np.zeros((P, width), np.float32)
    c = 0
    for v in vecs:
        v = np.asarray(v, np.float32).reshape(-1)
        nch = v.size // P
        out[:, c:c + nch] = v.reshape(nch, P).T
        c += nch
    assert c <= width
    return out


def _pack(inputs):
    obs = np.asarray(inputs["obs"], np.float32)
    action = np.asarray(inputs["action"])
    enc = {k: np.asarray(v, np.float32) for k, v in inputs["enc"].items()}
    edge = inputs["edge"]
    node = inputs["node"]

    def mlp_np(m):
        return {
            "w0": np.asarray(m["w0"], np.float32),
            "b0": np.asarray(m["b0"], np.float32),
            "hid": [{k: np.asarray(v, np.float32) for k, v in h.items()}
                    for h in m["hid"]],
            "wf": np.asarray(m["wf"], np.float32),
            "bf": np.asarray(m["bf"], np.float32),
        }

    edge = mlp_np(edge)
    node = mlp_np(node)

    d = {}
    # encoder L1 (10->512, LN): pad K 10->128
    w1p, b1p, s1 = _center_fold(enc["w1"], enc["b1"], enc["g1"])
    w1_pad = np.zeros((P, H), np.float32)
    w1_pad[:10] = w1p
    d["ew1"] = w1_pad
    w2p, b2p, s2 = _center_fold(enc["w2"], enc["b2"], enc["g2"])
    d["ew2"] = _kchunks(w2p)
    d["ew3"] = _kchunks(enc["w3"])
    d["cols"] = _colpack([
        b1p, s1, enc["bb1"],
        b2p, s2, enc["bb2"],
        enc["b3"],
        edge["b0"],
    ])
    # edge mlp
    d["gw0"] = _kchunks(edge["w0"])
    e1 = edge["hid"][0]
    gw1p, gb1p, gs1 = _center_fold(e1["w"], e1["b"], e1["g"])
    d["gw1"] = _kchunks(gw1p)
    e2 = edge["hid"][1]
    gw2p, gb2p, gs2 = _center_fold(e2["w"], e2["b"], e2["g"])
    d["gw2"] = _kchunks(gw2p)
    d["gwf"] = _kchunks(edge["wf"])
    n0b = node["b0"]
    d["cols2"] = _colpack([
        gb1p, gs1, e1["beta"],
        gb2p, gs2, e2["beta"],
        2.0 * edge["bf"],
        n0b,
    ])
    # node mlp: w0 rows: 0:128 state, 128:133 onehot, 133:645 agg
    nw0 = node["w0"]
    d["nw0s"] = np.ascontiguousarray(nw0[0:D])
    nw0a = np.zeros((P, A, H), np.float32)
    for a in range(A):
        nw0a[OH_OFF + a * ACT:OH_OFF + (a + 1) * ACT, a] = nw0[D:D + ACT]
    d["nw0a"] = nw0a
    d["nw0g"] = _kchunks(nw0[D + ACT:])
    h1n = node["hid"][0]
    nw1p, nb1p, ns1 = _center_fold(h1n["w"], h1n["b"], h1n["g"])
    d["nw1"] = _kchunks(nw1p)
    h2n = node["hid"][1]
    nw2p, nb2p, ns2 = _center_fold(h2n["w"], h2n["b"], h2n["g"])
    d["nw2"] = _kchunks(nw2p)
    d["nwf"] = _kchunks(node["wf"])
    d["cols3"] = _colpack([
        nb1p, ns1, h1n["beta"],
        nb2p, ns2, h2n["beta"],
        node["bf"],
    ])
    d["scols"] = _colpack([s1, s2, gs1, gs2, ns1, ns2], width=24)

    # io slab: [P, B]: rows 0:10 obs.T, rows OH_OFF..OH_OFF+15 one-hot
    nB = obs.shape[0]
    io = np.zeros((P, nB), np.float32)
    io[:10] = obs.T
    act = np.asarray(action).astype(np.int64)
    for a in range(A):
        for v in range(ACT):
            io[OH_OFF + a * ACT + v] = (act[:, a] == v).astype(np.float32)
    d["io_full"] = io
    return d


def _get_executor():
    """Build (once) a non-donating jitted shard_map executor over 8 cores.

    Returns (sharded_fn, in_names, out_names, out_avals).
    """
    if "exec" in _CACHE:
        return _CACHE["exec"]

    import jax
    from jax.sharding import Mesh, PartitionSpec
    from jax.experimental.shard_map import shard_map
    from concourse import bass2jax
    from concourse import mybir as _mb

    nc = _CACHE.get(("prog", T))
    if nc is None:
        nc = _build_program(T)
        _CACHE[("prog", T)] = nc

    bass2jax.install_neuronx_cc_hook()

    partition_name = (nc.partition_id_tensor.name
                      if nc.partition_id_tensor else None)
    in_names, out_names, out_avals = [], [], []
    for alloc in nc.m.functions[0].allocations:
        if not isinstance(alloc, _mb.MemoryLocationSet):
            continue
        name = alloc.memorylocations[0].name
        if alloc.kind == "ExternalInput":
            if name != partition_name:
                in_names.append(name)
        elif alloc.kind == "ExternalOutput":
            out_names.append(name)
            out_avals.append(jax.core.ShapedArray(tuple(alloc.tensor_shape),
                                                  _mb.dt.np(alloc.dtype)))

    prim_in_names = in_names + out_names
    if partition_name is not None:
        prim_in_names = prim_in_names + [partition_name]

    def _body(*args):
        operands = list(args)
        if partition_name is not None:
            operands.append(bass2jax.partition_id_tensor())
        outs = bass2jax._bass_exec_p.bind(
            *operands,
            out_avals=tuple(out_avals),
            in_names=tuple(prim_in_names),
            out_names=tuple(out_names),
            lowering_input_output_aliases=(),
            sim_require_finite=True,
            sim_require_nnan=True,
            nc=nc,
        )
        return tuple(outs)

    devices = jax.devices()[:N_CORES]
    mesh = Mesh(np.asarray(devices), ("core",))
    nin = len(in_names) + len(out_names)
    sharded = jax.jit(
        shard_map(_body, mesh=mesh,
                  in_specs=(PartitionSpec("core"),) * nin,
                  out_specs=(PartitionSpec("core"),) * len(out_names),
                  check_rep=False),
        keep_unused=True,
    )
    _CACHE["exec"] = (sharded, in_names, out_names, out_avals, mesh)
    return _CACHE["exec"]


def _device_args(inputs):
    """Pack inputs and return the concatenated global arg list for the executor."""
    sharded, in_names, out_names, out_avals, mesh = _get_executor()
    d = _pack(inputs)
    io = d.pop("io_full")
    args = []
    for name in in_names:
        if name == "io_p":
            # per-core slices stacked along axis 0
            a = np.concatenate(
                [io[:, c * BPC:(c + 1) * BPC] for c in range(N_CORES)], axis=0)
        else:
            a = np.concatenate([d[name]] * N_CORES, axis=0)
        args.append(np.ascontiguousarray(a))
    for av in out_avals:
        args.append(np.zeros((N_CORES * av.shape[0], *av.shape[1:]), av.dtype))
    return args


def _unpack_out(out_arrs):
    # out_arrs[0]: global [N_CORES*P, A, BPC]
    g = np.asarray(out_arrs[0]).reshape(N_CORES, P, A, BPC)
    # core c, [P, A, BPC] -> concat cores along samples
    out_fm = np.concatenate([g[c] for c in range(N_CORES)], axis=2)  # [P, A, B]
    return np.ascontiguousarray(out_fm.transpose(2, 1, 0)).astype(np.float32)


def kernel(**inputs) -> np.ndarray:
    sharded, in_names, out_names, out_avals, mesh = _get_executor()
    args = _device_args(inputs)
    out_arrs = sharded(*args)
    return _unpack_out(out_arrs)


# revision 9
# speedup vs baseline: 80886787.0000x; 80886787.0000x over previous
"""Trainium2 Bass kernel for the ContrastiveSWM transition model (GNN message passing).

Full-input contract: kernel(**inputs) takes the unsharded inputs from
reference.setup_inputs() and returns the full [16384, 3, 128] output.
Internally: batch is sharded across 8 NeuronCores (2048 samples each), params
replicated; each core runs an identical Bass/Tile program over 4 tiles of 512
samples kept feature-major ([feature, sample]) in SBUF.

Key implementation choices:
- All matmuls run as float32r (fp32 storage, fast PE path at N=512).
- LayerNorm mean-subtraction is folded into host-pre-centered weights
  (mean(y) over features is linear in x), LN gain g folded into weights;
  variance comes from a (1/g^2)-weighted ones-matmul over squared
  activations; 1/std is broadcast across partitions on GPSIMD.
- Per-sample 3-node graph: src/tgt gather = partition-row slices of the
  feature-major state; scatter-add over edges = accumulating both edges'
  final-layer matmul groups into one PSUM tile.
- one_hot(action) is precomputed on the host into a padded input slab.
"""

import os
import sys
from contextlib import ExitStack

import numpy as np

for _p in ("/opt/trn_rl_repo", "/root/.axon_site/_ro/trn_rl_repo"):
    if os.path.isdir(_p) and _p not in sys.path:
        sys.path.insert(0, _p)

import concourse.bass as bass  # noqa: E402
import concourse.mybir as mybir  # noqa: E402
import concourse.tile as tile  # noqa: E402
from concourse import bacc  # noqa: E402

P = 128
NT = 512          # samples per tile (matmul moving free dim)
N_CORES = 8
B = 16384
BPC = B // N_CORES            # 2048 samples per core
T = BPC // NT                 # 4 tiles per core
D = 128                       # latent dim per agent
H = 512                       # hidden
A = 3                         # agents
ACT = 5                       # action dim
EPS = 1e-5
F32 = mybir.dt.float32
F32R = mybir.dt.float32r
AF = mybir.ActivationFunctionType

# edge list: ROWS=src, COLS=tgt; edges 2a, 2a+1 share src agent a
EDGE_TGTS = {0: (1, 2), 1: (0, 2), 2: (0, 1)}

# one-hot rows live at OH_OFF + a*ACT + v inside the 128-row input slab
OH_OFF = 16

_CACHE = {}


def _build_program(n_tiles: int):
    nc = bacc.Bacc("TRN2", target_bir_lowering=False, debug=False)
    BT = n_tiles * NT

    def din(name, shape, dt=F32R):
        return nc.dram_tensor(name, list(shape), dt, kind="ExternalInput").ap()

    io_d = din("io_p", [P, BT])
    # encoder
    ew1_d = din("ew1", [P, H])
    ew2_d = din("ew2", [P, 4, H])
    ew3_d = din("ew3", [P, 4, A * D])
    # edge mlp
    gw0_d = din("gw0", [P, 2, H])
    gw1_d = din("gw1", [P, 4, H])
    gw2_d = din("gw2", [P, 4, H])
    gwf_d = din("gwf", [P, 4, H])
    # node mlp
    nw0s_d = din("nw0s", [P, H])
    nw0a_d = din("nw0a", [P, A, H])
    nw0g_d = din("nw0g", [P, 4, H])
    nw1_d = din("nw1", [P, 4, H])
    nw2_d = din("nw2", [P, 4, H])
    nwf_d = din("nwf", [P, 4, D])
    # packed per-chunk column vectors [P, ncols]
    cols_d = din("cols", [P, 32], F32)
    out_d = nc.dram_tensor("out_fm", [P, A, BT], F32, kind="ExternalOutput").ap()

    # column indices in cols_d (see _pack_cols)
    CI = {}
    _ci = iter(range(64))

    def ci(name, n):
        CI[name] = next(_ci)
        for _ in range(n - 1):
            next(_ci)
        return CI[name]

    ci("eb1", 4); ci("es1", 4); ci("ebt1", 4)
    ci("eb2", 4); ci("es2", 4); ci("ebt2", 4)
    ci("eb3", 3)
    ci("gb0", 4)  # 31 cols -> fits 32
    cols2_d = din("cols2", [P, 32], F32)
    CI2 = {}
    _ci2 = iter(range(64))

    def ci2(name, n):
        CI2[name] = next(_ci2)
        for _ in range(n - 1):
            next(_ci2)
        return CI2[name]

    ci2("gb1", 4); ci2("gs1", 4); ci2("gbt1", 4)
    ci2("gb2", 4); ci2("gs2", 4); ci2("gbt2", 4)
    ci2("gbf2", 4)
    ci2("nb0", 4)  # 32 cols exactly
    cols3_d = din("cols3", [P, 32], F32)
    scols_d = din("scols", [P, 24], mybir.dt.bfloat16)
    brow_d = din("brow", [1, 6 * H], F32R)
    ones_d = din("onesrow", [1, NT], F32R)
    CI3 = {}
    _ci3 = iter(range(64))

    def ci3(name, n):
        CI3[name] = next(_ci3)
        for _ in range(n - 1):
            next(_ci3)
        return CI3[name]

    ci3("nb1", 4); ci3("ns1", 4); ci3("nbt1", 4)
    ci3("nb2", 4); ci3("ns2", 4); ci3("nbt2", 4)
    ci3("nbf", 1)

    with tile.TileContext(nc) as tc:
        with ExitStack() as ctx:
            wp = ctx.enter_context(tc.tile_pool(name="wp", bufs=1))
            wk = ctx.enter_context(tc.tile_pool(name="wk", bufs=2))
            pmm = ctx.enter_context(tc.tile_pool(name="pmm", bufs=6, space="PSUM"))
            psq = ctx.enter_context(tc.tile_pool(name="psq", bufs=2, space="PSUM"))

            # ---- load weights (once) ----
            def wload(ap_d, shape, name, dt=F32R):
                t = wp.tile(shape, dt, name=name, tag=name)
                nc.sync.dma_start(t[:], ap_d[:])
                return t

            ew1 = wload(ew1_d, [P, H], "ew1")
            ew2 = wload(ew2_d, [P, 4, H], "ew2")
            ew3 = wload(ew3_d, [P, 4, A * D], "ew3")
            gw0 = wload(gw0_d, [P, 2, H], "gw0")
            gw1 = wload(gw1_d, [P, 4, H], "gw1")
            gw2 = wload(gw2_d, [P, 4, H], "gw2")
            gwf = wload(gwf_d, [P, 4, H], "gwf")
            nw0s = wload(nw0s_d, [P, H], "nw0s")
            nw0a = wload(nw0a_d, [P, A, H], "nw0a")
            nw0g = wload(nw0g_d, [P, 4, H], "nw0g")
            nw1 = wload(nw1_d, [P, 4, H], "nw1")
            nw2 = wload(nw2_d, [P, 4, H], "nw2")
            nwf = wload(nwf_d, [P, 4, D], "nwf")
            cols = wload(cols_d, [P, 32], "cols", F32)
            cols2 = wload(cols2_d, [P, 32], "cols2", F32)
            cols3 = wload(cols3_d, [P, 32], "cols3", F32)
            scols = wload(scols_d, [P, 24], "scols", mybir.dt.bfloat16)
            brow = wload(brow_d, [1, 6 * H], "brow", F32R)
            ones_row = wload(ones_d, [1, NT], "ones_row", F32R)
            eps_t = wp.tile([1, 1], F32, name="eps_t", tag="eps_t")
            nc.vector.memset(eps_t[:], EPS)

            def col(name):
                if name in CI:
                    return cols[:, CI[name]:CI[name] + 1]
                if name in CI2:
                    return cols2[:, CI2[name]:CI2[name] + 1]
                return cols3[:, CI3[name]:CI3[name] + 1]

            SCI = {"es1": 0, "es2": 4, "gs1": 8, "gs2": 12, "ns1": 16, "ns2": 20}
            BROW = {"eb1": 0, "eb2": H, "gb1": 2 * H, "gb2": 3 * H,
                    "nb1": 4 * H, "nb2": 5 * H}

            def scolq(name, m):
                i = SCI[name] + m
                return scols[:, i:i + 1]

            def colq(name, m):
                if name in CI:
                    return cols[:, CI[name] + m:CI[name] + m + 1]
                if name in CI2:
                    return cols2[:, CI2[name] + m:CI2[name] + m + 1]
                return cols3[:, CI3[name] + m:CI3[name] + m + 1]

            def mm_group(ps, w_chunks, x_chunks, start=True, stop=True):
                n = len(w_chunks)
                for k in range(n):
                    nc.tensor.matmul(
                        ps[:],
                        w_chunks[k].bitcast(F32R),
                        x_chunks[k].bitcast(F32R),
                        start=(start and k == 0),
                        stop=(stop and k == n - 1),
                    )

            def ln_relu_layer(x_chunks, w3, bname, sname, btname, out_tag, MC=4):
                """relu(LN(x @ W + b)) with host-pre-centered W; returns [P,MC,NT] quad.

                Bias lands in PSUM via a rank-1 ones-row matmul; squared
                activations are bf16; relu+beta alternates ACT/DVE."""
                out = wk.tile([P, MC, NT], F32R, name=f"{out_tag}", tag=out_tag)
                tq = wk.tile([P, MC, NT], F32, name="tq", tag="tq")
                sq = wk.tile([P, MC, NT], mybir.dt.bfloat16, name="sq", tag="sq")
                ssq = psq.tile([1, NT], F32, name="ssq", tag="ssq")
                pss = []
                boff = BROW[bname]
                for m in range(MC):
                    ps = pmm.tile([P, NT], F32, name="ps", tag="mm")
                    pss.append(ps)
                    nc.tensor.matmul(ps[:], brow[0:1, boff + m * P:boff + (m + 1) * P],
                                     ones_row[0:1, :], start=True, stop=False)
                    mm_group(ps, [w3[:, k, m * P:(m + 1) * P] for k in range(len(x_chunks))],
                             x_chunks, start=False, stop=True)
                    nc.scalar.activation(sq[:, m, :], ps[:], AF.Square)
                    nc.tensor.matmul(ssq[:], scolq(sname, m),
                                     sq[:, m, :],
                                     start=(m == 0), stop=(m == MC - 1))
                sd = wk.tile([1, NT], F32, name="sd", tag="sd")
                nc.scalar.activation(sd[:], ssq[:], AF.Sqrt, bias=eps_t[:],
                                     scale=1.0 / H)
                nc.vector.reciprocal(sd[:], sd[:])
                ibc = wk.tile([P, NT], F32, name="ibc", tag="ibc")
                nc.gpsimd.partition_broadcast(ibc[:], sd[:])
                for m in range(MC):
                    nc.vector.tensor_mul(out=tq[:, m, :], in0=pss[m][:], in1=ibc[:])
                    if m % 2 == 0:
                        nc.scalar.activation(out[:, m, :], tq[:, m, :], AF.Relu,
                                             bias=colq(btname, m), scale=1.0)
                    else:
                        nc.vector.tensor_scalar(out[:, m, :], tq[:, m, :],
                                                colq(btname, m), 0.0,
                                                mybir.AluOpType.add,
                                                mybir.AluOpType.max)
                return out

            def relu_layer(x_chunks, w_chunks_fn, bname, out_tag, MC=4):
                out = wk.tile([P, MC, NT], F32R, name=f"{out_tag}", tag=out_tag)
                for m in range(MC):
                    ps = pmm.tile([P, NT], F32, name="ps", tag="mm")
                    mm_group(ps, w_chunks_fn(m), x_chunks)
                    if m % 2 == 0:
                        nc.scalar.activation(out[:, m, :], ps[:], AF.Relu,
                                             bias=colq(bname, m), scale=1.0)
                    else:
                        nc.vector.tensor_scalar(out[:, m, :], ps[:],
                                                colq(bname, m), 0.0,
                                                mybir.AluOpType.add,
                                                mybir.AluOpType.max)
                return out

            # ---- per-tile pipeline ----
            for t in range(n_tiles):
                io = wk.tile([P, NT], F32R, name="io", tag="io")
                nc.sync.dma_start(io[:], io_d[:, t * NT:(t + 1) * NT])

                # encoder
                h1 = ln_relu_layer([io[:]], _W1View(ew1),
                                   "eb1", "es1", "ebt1", "qa")
                h2 = ln_relu_layer([h1[:, k, :] for k in range(4)], ew2,
                                   "eb2", "es2", "ebt2", "qb")
                state = wk.tile([P, A, NT], F32R, name="state", tag="state")
                for m in range(A):
                    ps = pmm.tile([P, NT], F32, name="ps", tag="mm")
                    mm_group(ps, [ew3[:, k, m * P:(m + 1) * P] for k in range(4)],
                             [h2[:, k, :] for k in range(4)])
                    nc.vector.tensor_scalar_add(state[:, m, :], ps[:], colq("eb3", m))

                for a in range(A):
                    # two edges with src == a; run both edge MLPs first, then
                    # accumulate both final layers into one contiguous PSUM group
                    e2s = []
                    for tgt in EDGE_TGTS[a]:
                        e0 = relu_layer(
                            [state[:, a, :], state[:, tgt, :]],
                            lambda m: [gw0[:, 0, m * P:(m + 1) * P],
                                       gw0[:, 1, m * P:(m + 1) * P]],
                            "gb0", "qa")
                        e1 = ln_relu_layer([e0[:, k, :] for k in range(4)], gw1,
                                           "gb1", "gs1", "gbt1", "qb")
                        e2s.append(ln_relu_layer([e1[:, k, :] for k in range(4)], gw2,
                                                 "gb2", "gs2", "gbt2", "qc"))
                    agg = wk.tile([P, 4, NT], F32R, name="agg", tag="agg", bufs=1)
                    for m in range(4):
                        agg_ps = pmm.tile([P, NT], F32, name="aggps", tag="mm")
                        for ei in range(2):
                            mm_group(agg_ps,
                                     [gwf[:, k, m * P:(m + 1) * P] for k in range(4)],
                                     [e2s[ei][:, k, :] for k in range(4)],
                                     start=(ei == 0), stop=(ei == 1))
                        nc.vector.tensor_scalar_add(agg[:, m, :], agg_ps[:],
                                                    colq("gbf2", m))

                    # node mlp for agent a
                    n0 = relu_layer(
                        [state[:, a, :], io[:], *(agg[:, k, :] for k in range(4))],
                        lambda m: [nw0s[:, m * P:(m + 1) * P],
                                   nw0a[:, a, m * P:(m + 1) * P],
                                   *(nw0g[:, k, m * P:(m + 1) * P] for k in range(4))],
                        "nb0", "qa")
                    n1 = ln_relu_layer([n0[:, k, :] for k in range(4)], nw1,
                                       "nb1", "ns1", "nbt1", "qb")
                    n2 = ln_relu_layer([n1[:, k, :] for k in range(4)], nw2,
                                       "nb2", "ns2", "nbt2", "qc")
                    ps = pmm.tile([P, NT], F32, name="ps", tag="mm")
                    mm_group(ps, [nwf[:, k, :] for k in range(4)],
                             [n2[:, k, :] for k in range(4)])
                    outp = wk.tile([P, NT], F32, name="outp", tag="outp")
                    nc.vector.tensor_add(out=outp[:], in0=ps[:], in1=state[:, a, :])
                    nc.vector.tensor_scalar_add(outp[:], outp[:], col("nbf"))
                    nc.sync.dma_start(out_d[:, a, t * NT:(t + 1) * NT], outp[:])

    nc.compile()
    return nc


class _W1View:
    """Adapter so ew1 [P, H] can be indexed like a [P, 1, H] 3-tensor."""

    def __init__(self, t):
        self.t = t

    def __getitem__(self, idx):
        _, k, sl = idx
        assert k == 0
        return self.t[:, sl]


def _center_fold(W, b, g):
    Wp = (W - W.mean(axis=1, keepdims=True)) * g[None, :]
    bp = (b - b.mean()) * g
    s = 1.0 / (g * g)
    return Wp.astype(np.float32), bp.astype(np.float32), s.astype(np.float32)


def _kchunks(W):
    K, M = W.shape
    assert K % P == 0
    return np.ascontiguousarray(W.reshape(K // P, P, M).transpose(1, 0, 2))


def _colpack(vecs, width=32):
    """Pack a list of [M] vectors (M multiple of 128) into [P, width]."""
    out = np.zeros((P, width), np.float32)
    c = 0
    for v in vecs:
        v = np.asarray(v, np.float32).reshape(-1)
        nch = v.size // P
        out[:, c:c + nch] = v.reshape(nch, P).T
        c += nch
    assert c <= width
    return out


def _pack(inputs):
    obs = np.asarray(inputs["obs"], np.float32)
    action = np.asarray(inputs["action"])
    enc = {k: np.asarray(v, np.float32) for k, v in inputs["enc"].items()}
    edge = inputs["edge"]
    node = inputs["node"]

    def mlp_np(m):
        return {
            "w0": np.asarray(m["w0"], np.float32),
            "b0": np.asarray(m["b0"], np.float32),
            "hid": [{k: np.asarray(v, np.float32) for k, v in h.items()}
                    for h in m["hid"]],
            "wf": np.asarray(m["wf"], np.float32),
            "bf": np.asarray(m["bf"], np.float32),
        }

    edge = mlp_np(edge)
    node = mlp_np(node)

    d = {}
    # encoder L1 (10->512, LN): pad K 10->128
    w1p, b1p, s1 = _center_fold(enc["w1"], enc["b1"], enc["g1"])
    w1_pad = np.zeros((P, H), np.float32)
    w1_pad[:10] = w1p
    d["ew1"] = w1_pad
    w2p, b2p, s2 = _center_fold(enc["w2"], enc["b2"], enc["g2"])
    d["ew2"] = _kchunks(w2p)
    d["ew3"] = _kchunks(enc["w3"])
    d["cols"] = _colpack([
        b1p, s1, enc["bb1"],
        b2p, s2, enc["bb2"],
        enc["b3"],
        edge["b0"],
    ])
    # edge mlp
    d["gw0"] = _kchunks(edge["w0"])
    e1 = edge["hid"][0]
    gw1p, gb1p, gs1 = _center_fold(e1["w"], e1["b"], e1["g"])
    d["gw1"] = _kchunks(gw1p)
    e2 = edge["hid"][1]
    gw2p, gb2p, gs2 = _center_fold(e2["w"], e2["b"], e2["g"])
    d["gw2"] = _kchunks(gw2p)
    d["gwf"] = _kchunks(edge["wf"])
    n0b = node["b0"]
    d["cols2"] = _colpack([
        gb1p, gs1, e1["beta"],
        gb2p, gs2, e2["beta"],
        2.0 * edge["bf"],
        n0b,
    ])
    # node mlp: w0 rows: 0:128 state, 128:133 onehot, 133:645 agg
    nw0 = node["w0"]
    d["nw0s"] = np.ascontiguousarray(nw0[0:D])
    nw0a = np.zeros((P, A, H), np.float32)
    for a in range(A):
        nw0a[OH_OFF + a * ACT:OH_OFF + (a + 1) * ACT, a] = nw0[D:D + ACT]
    d["nw0a"] = nw0a
    d["nw0g"] = _kchunks(nw0[D + ACT:])
    h1n = node["hid"][0]
    nw1p, nb1p, ns1 = _center_fold(h1n["w"], h1n["b"], h1n["g"])
    d["nw1"] = _kchunks(nw1p)
    h2n = node["hid"][1]
    nw2p, nb2p, ns2 = _center_fold(h2n["w"], h2n["b"], h2n["g"])
    d["nw2"] = _kchunks(nw2p)
    d["nwf"] = _kchunks(node["wf"])
    d["cols3"] = _colpack([
        nb1p, ns1, h1n["beta"],
        nb2p, ns2, h2n["beta"],
        node["bf"],
    ])
    import ml_dtypes
    d["scols"] = _colpack([s1, s2, gs1, gs2, ns1, ns2],
                          width=24).astype(ml_dtypes.bfloat16)
    d["brow"] = np.concatenate(
        [b1p, b2p, gb1p, gb2p, nb1p, nb2p]).astype(np.float32).reshape(1, 6 * H)
    d["onesrow"] = np.ones((1, NT), np.float32)

    # io slab: [P, B]: rows 0:10 obs.T, rows OH_OFF..OH_OFF+15 one-hot
    nB = obs.shape[0]
    io = np.zeros((P, nB), np.float32)
    io[:10] = obs.T
    act = np.asarray(action).astype(np.int64)
    for a in range(A):
        for v in range(ACT):
            io[OH_OFF + a * ACT + v] = (act[:, a] == v).astype(np.float32)
    d["io_full"] = io
    return d


def _get_executor():
    """Build (once) a non-donating jitted shard_map executor over 8 cores.

    Returns (sharded_fn, in_names, out_names, out_avals).
    """
    if "exec" in _CACHE:
        return _CACHE["exec"]

    import jax
    from jax.sharding import Mesh, PartitionSpec
    from jax.experimental.shard_map import shard_map
    from concourse import bass2jax
    from concourse import mybir as _mb

    nc = _CACHE.get(("prog", T))
    if nc is None:
        nc = _build_program(T)
        _CACHE[("prog", T)] = nc

    bass2jax.install_neuronx_cc_hook()

    partition_name = (nc.partition_id_tensor.name
                      if nc.partition_id_tensor else None)
    in_names, out_names, out_avals = [], [], []
    for alloc in nc.m.functions[0].allocations:
        if not isinstance(alloc, _mb.MemoryLocationSet):
            continue
        name = alloc.memorylocations[0].name
        if alloc.kind == "ExternalInput":
            if name != partition_name:
                in_names.append(name)
        elif alloc.kind == "ExternalOutput":
            out_names.append(name)
            out_avals.append(jax.core.ShapedArray(tuple(alloc.tensor_shape),
                                                  _mb.dt.np(alloc.dtype)))

    prim_in_names = in_names + out_names
    if partition_name is not None:
        prim_in_names = prim_in_names + [partition_name]

    def _body(*args):
        operands = list(args)
        if partition_name is not None:
            operands.append(bass2jax.partition_id_tensor())
        outs = bass2jax._bass_exec_p.bind(
            *operands,
            out_avals=tuple(out_avals),
            in_names=tuple(prim_in_names),
            out_names=tuple(out_names),
            lowering_input_output_aliases=(),
            sim_require_finite=True,
            sim_require_nnan=True,
            nc=nc,
        )
        return tuple(outs)

    devices = jax.devices()[:N_CORES]
    mesh = Mesh(np.asarray(devices), ("core",))
    nin = len(in_names) + len(out_names)
    sharded = jax.jit(
        shard_map(_body, mesh=mesh,
                  in_specs=(PartitionSpec("core"),) * nin,
                  out_specs=(PartitionSpec("core"),) * len(out_names),
                  check_rep=False),
        keep_unused=True,
    )
    _CACHE["exec"] = (sharded, in_names, out_names, out_avals, mesh)
    return _CACHE["exec"]


def _device_args(inputs):
    """Pack inputs and return the concatenated global arg list for the executor."""
    sharded, in_names, out_names, out_avals, mesh = _get_executor()
    d = _pack(inputs)
    io = d.pop("io_full")
    args = []
    for name in in_names:
        if name == "io_p":
            # per-core slices stacked along axis 0
            a = np.concatenate(
                [io[:, c * BPC:(c + 1) * BPC] for c in range(N_CORES)], axis=0)
        else:
            a = np.concatenate([d[name]] * N_CORES, axis=0)
        args.append(np.ascontiguousarray(a))
    for av in out_avals:
        args.append(np.zeros((N_CORES * av.shape[0], *av.shape[1:]), av.dtype))
    return args


def _unpack_out(out_arrs):
    # out_arrs[0]: global [N_CORES*P, A, BPC]
    g = np.asarray(out_arrs[0]).reshape(N_CORES, P, A, BPC)
    # core c, [P, A, BPC] -> concat cores along samples
    out_fm = np.concatenate([g[c] for c in range(N_CORES)], axis=2)  # [P, A, B]
    return np.ascontiguousarray(out_fm.transpose(2, 1, 0)).astype(np.float32)


def kernel(**inputs) -> np.ndarray:
    sharded, in_names, out_names, out_avals, mesh = _get_executor()
    args = _device_args(inputs)
    out_arrs = sharded(*args)
    return _unpack_out(out_arrs)
